# revision 38
# baseline (speedup 1.0000x reference)
"""Trainium2 Bass kernel for MultiHeadDilatedAttention.

Full inputs in, full output out. Sharding: 8 cores = (batch b in 0..3) x
(segment-position half). Each (b, s) pair is an independent attention problem
(attention runs across segments n at fixed position-in-segment s), so each
core handles b = c//2 and 64 of the 128 s values. No collectives needed: the
output rows t = s*64 + dil*l for a core's s-range form a contiguous chunk of
y[b].

v2 layout (all matmuls bf16 with fp32 PSUM accumulation):
  x cast to bf16 + transposed on host -> [ec, blk, 128, 1024] in DRAM
  pipelined per 16-segment block: DMA x block, project Q^T/K^T/V^T
  V^T -> PE-transpose -> V natural [n slots, dv]
  KQ psum initialized with identity@mask matmul, then K^T_s x Q^T_s per s
  softmax: exp on Scalar (PSUM in), reduce+recip+mul on DVE
  att^T[v,m] = V_s-matmul(smKQ); scattered into class-blocked atT
  out-proj per offset-class chunk (only participating heads), y stored bf16,
  upcast + b_out added on host. PSUM->SBUF copies round-robin over
  Vector/Scalar/GpSimd.
"""

from contextlib import ExitStack

import numpy as np
import ml_dtypes

import concourse.bass as bass
import concourse.mybir as mybir
import concourse.tile as tile
from concourse import bacc
from concourse.masks import make_identity
from concourse.bass_utils import run_bass_kernel_spmd

F32 = mybir.dt.float32
BF16 = mybir.dt.bfloat16
AX = mybir.AxisListType
EXP = mybir.ActivationFunctionType.Exp

B, T, E = 4, 8192, 1024
SEG = 128          # segment size (= #s positions overall)
NB = T // SEG      # 64 segments
NS = 64            # s values per core
ROWS = NB * NS     # 4096 rows per core
DK = 128
H = 4
DILS = [1, 2, 4, 8]
LS = [NB // d for d in DILS]       # [64, 32, 16, 8]
SLOTL = [max(l, 32) for l in LS]   # partition slot stride: [64, 32, 32, 32]
G = [128 // sl for sl in SLOTL]    # s-slots per 128 partitions: [2, 4, 4, 4]
MG = [8, 16, 16, 16]               # m-groups (free) per KQ psum tile
NORM = float(1.0 / np.sqrt(DK))
NEG = -1.0e10
NECHUNK = E // 128                 # 8
NBLK = 4                           # x pipeline blocks
BLKN = NB // NBLK                  # 16 segments per block
NGRP = [NS // g for g in G]        # vnat groups per head: [32, 16, 16, 16]
NGT = 4                            # groups per attention tile (4*128 cols)
MQOFF = [0, 512, 1024, 1536]       # mask col offset per head (stacked masks)
MTOT = 2048

# out-projection classes: offsets o in [0,64) grouped by which heads hit them.
def _classes():
    out = []
    o_all = list(range(64))
    out.append(([o for o in o_all if o % 2 == 1], [0]))           # odd
    out.append(([o for o in o_all if o % 4 == 2], [0, 1]))        # 2 mod 4
    out.append(([o for o in o_all if o % 8 == 4], [0, 1, 2]))     # 4 mod 8
    out.append(([o for o in o_all if o % 8 == 0], [0, 1, 2, 3]))  # 0 mod 8
    return out


CLASSES = _classes()
# atT[h] column layout: per-class blocks (so out-proj lhsT slices are
# contiguous). HEAD_BLOCKS[h] = [(cid, l_list)], HEAD_OFF[h][cid] = col offset.
HEAD_BLOCKS = {}
HEAD_OFF = {}
for _h in range(H):
    blocks, offs, off = [], {}, 0
    for _cid, (_ol, _heads) in enumerate(CLASSES):
        if _h in _heads:
            ll = [o // DILS[_h] for o in _ol]
            blocks.append((_cid, ll))
            offs[_cid] = off
            off += len(ll) * NS
    HEAD_BLOCKS[_h] = blocks
    HEAD_OFF[_h] = offs
    assert off == LS[_h] * NS


def build_program(max_phase: int = 5) -> bass.Bass:
    nc = bacc.Bacc("TRN2", target_bir_lowering=False, debug=False)
    xs = nc.dram_tensor("xs", [NECHUNK, NBLK, 128, BLKN * NS], BF16,
                        kind="ExternalInput").ap()
    wqkv = nc.dram_tensor("wqkv", [12, 128, NECHUNK * 128], BF16,
                          kind="ExternalInput").ap()
    wout = nc.dram_tensor("wout", [128, H * E], BF16, kind="ExternalInput").ap()
    maskd = nc.dram_tensor("masks", [128, MTOT], BF16, kind="ExternalInput").ap()
    y = nc.dram_tensor("y", [ROWS, E], BF16, kind="ExternalOutput").ap()
    _build_phases(nc, max_phase, xs, wqkv, wout, maskd, y)
    nc.finalize()
    return nc


def _build_phases(nc, max_phase, xs, wqkv, wout, maskd, y):
    # round-robin engine copy (PSUM->SBUF traffic spread over 3 engines)
    state = {"i": 0}

    def rr_copy(out_ap, in_ap, engs=(0, 1)):
        # GPSIMD cannot access PSUM, so PSUM->SBUF traffic only has DVE+Act
        k = engs[state["i"] % len(engs)]
        state["i"] += 1
        if k == 0:
            nc.vector.tensor_copy(out=out_ap, in_=in_ap)
        elif k == 1:
            nc.scalar.copy(out_ap, in_ap)
        else:
            nc.gpsimd.tensor_copy(out=out_ap, in_=in_ap)

    with ExitStack() as ctx:
        tc = ctx.enter_context(tile.TileContext(nc))

        persist = ctx.enter_context(tc.tile_pool(name="persist", bufs=1))
        ident = persist.tile([128, 128], BF16, tag="ident")
        make_identity(nc, ident)

        # DMA descriptor generation costs ~600ns per dma_start and serializes
        # on the issuing engine's queue — spread the startup loads over the
        # idle engines so transfers begin ASAP
        dma_engs = [nc.sync, nc.scalar, nc.gpsimd]

        # QKV weights for h0 first (unblocks the first projection matmuls)
        w_sb = {}
        for h in range(H):
            for p in range(3):
                w_sb[(h, p)] = persist.tile([128, NECHUNK * 128], BF16,
                                            tag=f"w{h}{p}", name=f"w{h}{p}")
        for p in range(3):
            dma_engs[p].dma_start(out=w_sb[(0, p)], in_=wqkv[p, :, :])

        xt_pool = ctx.enter_context(tc.tile_pool(name="xt", bufs=2))

        def load_block(blk, engs):
            xt = [xt_pool.tile([128, BLKN * NS], BF16, tag=f"xt{ec}",
                               name=f"xt{ec}")
                  for ec in range(NECHUNK)]
            for ec in range(NECHUNK):
                engs[ec % len(engs)].dma_start(out=xt[ec], in_=xs[ec, blk, :, :])
            return xt

        xt0 = load_block(0, dma_engs)
        for h in range(1, H):
            for p in range(3):
                dma_engs[(h * 3 + p) % 3].dma_start(
                    out=w_sb[(h, p)], in_=wqkv[h * 3 + p, :, :])

        # warm the PE clock gate (HAM) during the initial DMA wait: ~3.5us of
        # dummy transposes flips the PE to 2.4 GHz before real work arrives
        with ExitStack() as wctx:
            warm_ps = wctx.enter_context(
                tc.tile_pool(name="warm_ps", bufs=1, space="PSUM"))
            wps = warm_ps.tile([128, 128], BF16, tag="warm")
            for _ in range(28):
                nc.tensor.transpose(wps, ident, ident)

        # persistent per-head tensors: Q^T/K^T/V^T all stored s-major with the
        # 32-padded slot stride (col = s*SLOTL + l) so a slot-group slice is
        # one contiguous 128-col window. Pad cols are zeroed so dead slots
        # contribute exact zeros through the stacked attention contraction.
        qkvpool = ctx.enter_context(tc.tile_pool(name="qkv", bufs=1))
        qkv_sb = {}
        for h in range(H):
            for p in range(3):
                ncol = SLOTL[h] * NS
                qkv_sb[(h, p)] = qkvpool.tile([128, ncol], BF16,
                                              tag=f"qkv{h}{p}", name=f"qkv{h}{p}")
                if LS[h] < SLOTL[h]:
                    pad = qkv_sb[(h, p)].rearrange(
                        "p (s l) -> p s l", l=SLOTL[h])[:, :, LS[h]:]
                    [nc.vector, nc.gpsimd, nc.vector][p].memset(pad, 0.0)

        # ------------- phase A+B: pipelined x load + QKV projection --------
        with ExitStack() as pctx:
            pj_ps = pctx.enter_context(
                tc.tile_pool(name="pj_ps", bufs=6, space="PSUM"))
            for blk in range(NBLK):
                xt = xt0 if blk == 0 else load_block(blk, [nc.gpsimd])
                if max_phase < 2:
                    continue
                for h in range(H):
                    dil, L = DILS[h], LS[h]
                    ln = BLKN // dil          # l values in this block
                    l0 = blk * ln
                    for p in range(3):
                        ncols = ln * NS
                        for t0 in range(0, ncols, 512):
                            tcols = min(512, ncols - t0)
                            lt, lcnt = t0 // NS, tcols // NS
                            ps = pj_ps.tile([128, tcols], F32)
                            for ec in range(NECHUNK):
                                lhsT = w_sb[(h, p)][:, ec * 128:(ec + 1) * 128]
                                # moving operand streamed (s, l)-ordered so
                                # the psum tile comes out s-major and the
                                # copy below has contiguous l-runs both sides
                                rhs = xt[ec].rearrange(
                                    "p (l j s) -> p s l j", j=dil, s=NS
                                )[:, :, lt:lt + lcnt, 0]
                                nc.tensor.matmul(ps, lhsT, rhs,
                                                 start=(ec == 0),
                                                 stop=(ec == NECHUNK - 1))
                            out_ap = qkv_sb[(h, p)].rearrange(
                                "p (s l) -> p s l", l=SLOTL[h]
                            )[:, :, l0 + lt:l0 + lt + lcnt]
                            in_ap = ps.rearrange("p (s l) -> p s l", l=lcnt)
                            rr_copy(out_ap, in_ap)

        # weights/mask needed from phase D/E on — queue their DMAs last
        wout_sb = persist.tile([128, H * E], BF16, tag="wout_sb")
        nc.sync.dma_start(out=wout_sb, in_=wout)
        mask_sb = persist.tile([128, MTOT], BF16, tag="mask_sb")
        nc.scalar.dma_start(out=mask_sb, in_=maskd)

        # ---------------- phase C: V^T -> V natural ------------------------
        if max_phase < 3:
            return
        atpool = ctx.enter_context(tc.tile_pool(name="atT", bufs=1))
        # class-blocked layout, s-major: col = HEAD_OFF[h][cid] + s*n_o + oidx
        atT = [atpool.tile([128, LS[h] * NS], BF16, tag=f"atT{h}",
                           name=f"atT{h}") for h in range(H)]
        vnpool = ctx.enter_context(tc.tile_pool(name="vnat", bufs=1))
        # vnat[h]: [128, ngroups, 128]; group gi holds V_s for s = gi*G[h]..+G[h]
        # at partition slots (s % G[h]) * SLOTL[h]
        vnat = [vnpool.tile([128, NGRP[h], 128], BF16, tag=f"vnat{h}",
                            name=f"vnat{h}") for h in range(H)]
        # with the 32-padded s-major V^T layout every vnat group is one
        # 128-col window transpose
        with ExitStack() as pctx:
            vt_ps = pctx.enter_context(
                tc.tile_pool(name="vt_ps", bufs=4, space="PSUM"))
            for h in (3, 2, 1, 0):
                vt = qkv_sb[(h, 2)]
                for gi in range(NGRP[h]):
                    pt = vt_ps.tile([128, 128], BF16)
                    nc.tensor.transpose(pt, vt[:, gi * 128:(gi + 1) * 128],
                                        ident)
                    rr_copy(vnat[h][:, gi, :], pt)

        # -------- phase D: attention per head (+ phase E interleaved) ------
        if max_phase < 4:
            return
        with ExitStack() as pctx:
            kq_ps = pctx.enter_context(
                tc.tile_pool(name="kq_ps", bufs=3, space="PSUM"))
            at_ps = pctx.enter_context(
                tc.tile_pool(name="at_ps", bufs=2, space="PSUM"))
            y_ps = pctx.enter_context(
                tc.tile_pool(name="y_ps", bufs=3, space="PSUM"))
            sm_pool = pctx.enter_context(tc.tile_pool(name="sm", bufs=3))
            small = pctx.enter_context(tc.tile_pool(name="small", bufs=4))
            yo_pool = pctx.enter_context(tc.tile_pool(name="y_sb", bufs=3))

            def emit_out(s0b, swid):
                # output projection for s range [s0b, s0b+swid)
                # atT block layout is s-major (col = OFF + s*n_o + oidx) so
                # the 128-col lhsT slice is contiguous (1 free dim)
                for cid, (ol, heads) in enumerate(CLASSES):
                    n_o = len(ol)
                    sc = 128 // n_o
                    for s0 in range(s0b, s0b + swid, sc):
                        # two half-width psum tiles; the PSUM->SBUF copies run
                        # on DVE and Scalar concurrently
                        ps_h = [y_ps.tile([128, 512], F32, tag="y",
                                          name=f"psy{half}")
                                for half in range(2)]
                        for hi, h in enumerate(heads):
                            c0 = HEAD_OFF[h][cid] + s0 * n_o
                            lhsT = atT[h][:, c0:c0 + 128]
                            for half in range(2):
                                cs = half * 512
                                nc.tensor.matmul(
                                    ps_h[half], lhsT,
                                    wout_sb[:, h * E + cs:h * E + cs + 512],
                                    start=(hi == 0), stop=(hi == len(heads) - 1))
                        y_sb = yo_pool.tile([128, E], BF16)
                        nc.vector.tensor_copy(out=y_sb[:, 0:512], in_=ps_h[0])
                        nc.scalar.copy(y_sb[:, 512:1024], ps_h[1])
                        do = ol[1] - ol[0] if n_o > 1 else 1
                        dst = y.rearrange("(s o) e -> s o e", o=64)[
                            s0:s0 + sc, ol[0]:ol[-1] + 1:do, :]
                        nc.sync.dma_start(out=dst, in_=y_sb)

            # group-stacked attention: one tile = NGT vnat-groups. Per group
            # ONE KQ matmul (stationary = K for all g slot-s side by side,
            # moving = Q likewise; block-diagonal mask kills cross terms) and
            # ONE att matmul (full-128 contraction; dead slots contribute
            # exact zeros). Column layout: q*128 + j*SLOTL + m.
            def emit_kq(h, gi0):
                kt, qt = qkv_sb[(h, 1)], qkv_sb[(h, 0)]
                ps_kq = kq_ps.tile([128, NGT * 128], F32, tag="kq")
                nc.tensor.matmul(
                    ps_kq, ident, mask_sb[:, MQOFF[h]:MQOFF[h] + NGT * 128],
                    start=True, stop=False)
                for q in range(NGT):
                    c0 = (gi0 + q) * 128
                    nc.tensor.matmul(
                        ps_kq[:, q * 128:(q + 1) * 128],
                        kt[:, c0:c0 + 128], qt[:, c0:c0 + 128],
                        start=False, stop=True)
                return ps_kq

            def emit_sm(h, gi0, ps_kq):
                enumer = sm_pool.tile([128, NGT * 128], BF16, tag="enumer")
                nc.scalar.activation(enumer, ps_kq, EXP, scale=NORM)
                sums = small.tile([128, NGT], F32, tag="sums")
                nc.vector.reduce_sum(
                    sums, enumer.rearrange("p (q c) -> p q c", c=128),
                    axis=AX.X)
                recip = small.tile([128, NGT], F32, tag="recip")
                nc.vector.reciprocal(recip, sums)
                smkq = sm_pool.tile([128, NGT * 128], BF16, tag="smkq")
                rc_bc = bass.AP(tensor=recip.tensor, offset=recip.offset,
                                ap=[recip.ap[0], [1, NGT], [0, 128]])
                nc.gpsimd.tensor_mul(smkq, enumer, rc_bc)
                return smkq

            def emit_att(h, gi0, smkq):
                g, sl = G[h], SLOTL[h]
                ps_at = at_ps.tile([128, NGT * 128], F32)
                for q in range(NGT):
                    nc.tensor.matmul(ps_at[:, q * 128:(q + 1) * 128],
                                     vnat[h][:, gi0 + q, :],
                                     smkq[:, q * 128:(q + 1) * 128],
                                     start=True, stop=True)
                # scatter into class-blocked atT[h] (s-major blocks); the
                # (q, j) dims merge: col = s_rel*SLOTL + m, s_rel in [0,NGT*g)
                ps_sm = ps_at.rearrange("p (s m) -> p s m", m=sl)
                ns_t = NGT * g
                for cid, ll in HEAD_BLOCKS[h]:
                    dl = ll[1] - ll[0] if len(ll) > 1 else 1
                    src = ps_sm[:, :, ll[0]:ll[-1] + 1:dl]
                    off = HEAD_OFF[h][cid]
                    nl = len(ll)
                    s0 = gi0 * g
                    dst = atT[h][:, off:off + NS * nl].rearrange(
                        "p (s x) -> p s x", x=nl)[:, s0:s0 + ns_t, :]
                    nc.vector.tensor_copy(out=dst, in_=src)

            batches = [(hh, gi0) for hh in (3, 2, 1, 0)
                       for gi0 in range(0, NGRP[hh], NGT)]
            if max_phase < 5:
                for h, gi0 in batches:
                    emit_sm(h, gi0, emit_kq(h, gi0))
            else:
                # 2-deep software pipeline: at step i emit KQ(i), then
                # softmax(i-1), then att+out-proj(i-2) — the PE always has
                # ~2 batches of KQ queued while a softmax is in flight
                kqs, sms = {}, {}
                n = len(batches)
                for i in range(n + 2):
                    if i < n:
                        h, gi0 = batches[i]
                        kqs[i] = emit_kq(h, gi0)
                    if 1 <= i < n + 1:
                        h, gi0 = batches[i - 1]
                        sms[i - 1] = emit_sm(h, gi0, kqs.pop(i - 1))
                    if i >= 2:
                        h, gi0 = batches[i - 2]
                        emit_att(h, gi0, sms.pop(i - 2))
                        # h0 tiles cover 8 s each; out-proj fires per 16
                        if h == 0 and (gi0 + NGT) % 8 == 0:
                            emit_out((gi0 - NGT) * G[0], 16)


_NC = None


def _get_program():
    global _NC
    if _NC is None:
        _NC = build_program()
    return _NC


def _host_inputs(Wk, Wq, Wv, W_out, b_out):
    bf = ml_dtypes.bfloat16
    Wstack = np.stack([Wq, Wk, Wv], 1)                     # [H, 3, 128, 1024]
    tmp = Wstack.reshape(H, 3, 128, NECHUNK, 128)          # [h, p, c, ec, r]
    wqkv_sb = np.ascontiguousarray(
        tmp.transpose(0, 1, 4, 3, 2)).reshape(12, 128, NECHUNK * 128
                                              ).astype(bf)
    wout_sb = np.ascontiguousarray(
        W_out.reshape(E, H, 128).transpose(2, 1, 0)).reshape(128, H * E
                                                             ).astype(bf)
    # stacked-group mask per head: [128, 128] with row p = k*SLOTL + n and
    # col c = j*SLOTL + m: keep iff j == k and m <= n (softmax runs over the
    # query axis m); cross-group blocks and dead slots are masked. Dead rows
    # (n >= L) get a diagonal escape on a dead column so their softmax sum is
    # 1 (not 0 -> inf recip -> NaN through the full-128 att contraction).
    mask_host = np.full((128, MTOT), NEG, np.float32)
    for h in range(H):
        L, sl = LS[h], SLOTL[h]
        base = np.full((128, 128), NEG, np.float32)
        for p in range(128):
            k, n = p // sl, p % sl
            if n < L:
                base[p, k * sl:k * sl + n + 1] = 0.0
            else:
                base[p, p] = 0.0
        for q in range(NGT):
            c0 = MQOFF[h] + q * 128
            mask_host[:, c0:c0 + 128] = base
    return wqkv_sb, wout_sb, mask_host.astype(bf)


def _shard_x(xbf, c):
    b, half = c // 2, c % 2
    xs = xbf[b].reshape(NB, SEG, E)[:, half * NS:(half + 1) * NS, :]
    xs = xs.reshape(ROWS, E)                       # rows (n, s)
    # device layout: [e-chunk, blk, e-in-chunk, row] (x^T per 128-wide e chunk)
    return np.ascontiguousarray(
        xs.reshape(NBLK, BLKN * NS, NECHUNK, 128).transpose(2, 0, 3, 1))


def _prepare(x, Wk, Wq, Wv, W_out, b_out):
    xbf = np.asarray(x, np.float32).astype(ml_dtypes.bfloat16)
    wqkv_sb, wout_sb, mask_host = _host_inputs(
        np.asarray(Wk, np.float32), np.asarray(Wq, np.float32),
        np.asarray(Wv, np.float32), np.asarray(W_out, np.float32),
        np.asarray(b_out, np.float32))
    in_maps = []
    for c in range(8):
        in_maps.append({"xs": _shard_x(xbf, c), "wqkv": wqkv_sb,
                        "wout": wout_sb, "masks": mask_host})
    return _get_program(), in_maps


def _gather(res, b_out):
    y = np.empty((B, T, E), np.float32)
    for c in range(8):
        b, half = c // 2, c % 2
        y[b, half * ROWS:(half + 1) * ROWS, :] = \
            res.results[c]["y"].astype(np.float32)
    y += np.asarray(b_out, np.float32).reshape(1, 1, E)
    return y


def kernel(x, Wk, Wq, Wv, W_out, b_out):
    nc, in_maps = _prepare(x, Wk, Wq, Wv, W_out, b_out)
    res = run_bass_kernel_spmd(nc, in_maps, core_ids=list(range(8)))
    return _gather(res, b_out)


# revision 39
# speedup vs baseline: 1.5124x; 1.5124x over previous
"""Trainium2 Bass kernel for MultiHeadDilatedAttention.

Full inputs in, full output out. Sharding: 8 cores = (batch b in 0..3) x
(segment-position half). Each (b, s) pair is an independent attention problem
(attention runs across segments n at fixed position-in-segment s), so each
core handles b = c//2 and 64 of the 128 s values. No collectives needed: the
output rows t = s*64 + dil*l for a core's s-range form a contiguous chunk of
y[b].

v2 layout (all matmuls bf16 with fp32 PSUM accumulation):
  x cast to bf16 + transposed on host -> [ec, blk, 128, 1024] in DRAM
  pipelined per 16-segment block: DMA x block, project Q^T/K^T/V^T
  V^T -> PE-transpose -> V natural [n slots, dv]
  KQ psum initialized with identity@mask matmul, then K^T_s x Q^T_s per s
  softmax: exp on Scalar (PSUM in), reduce+recip+mul on DVE
  att^T[v,m] = V_s-matmul(smKQ); scattered into class-blocked atT
  out-proj per offset-class chunk (only participating heads), y stored bf16,
  upcast + b_out added on host. PSUM->SBUF copies round-robin over
  Vector/Scalar/GpSimd.
"""

from contextlib import ExitStack

import numpy as np
import ml_dtypes

import concourse.bass as bass
import concourse.mybir as mybir
import concourse.tile as tile
from concourse import bacc
from concourse.masks import make_identity
from concourse.bass_utils import run_bass_kernel_spmd

F32 = mybir.dt.float32
BF16 = mybir.dt.bfloat16
AX = mybir.AxisListType
EXP = mybir.ActivationFunctionType.Exp

B, T, E = 4, 8192, 1024
SEG = 128          # segment size (= #s positions overall)
NB = T // SEG      # 64 segments
NS = 64            # s values per core
ROWS = NB * NS     # 4096 rows per core
DK = 128
H = 4
DILS = [1, 2, 4, 8]
LS = [NB // d for d in DILS]       # [64, 32, 16, 8]
SLOTL = [max(l, 32) for l in LS]   # partition slot stride: [64, 32, 32, 32]
G = [128 // sl for sl in SLOTL]    # s-slots per 128 partitions: [2, 4, 4, 4]
MG = [8, 16, 16, 16]               # m-groups (free) per KQ psum tile
NORM = float(1.0 / np.sqrt(DK))
NEG = -1.0e10
NECHUNK = E // 128                 # 8
NBLK = 4                           # x pipeline blocks
BLKN = NB // NBLK                  # 16 segments per block
NGRP = [NS // g for g in G]        # vnat groups per head: [32, 16, 16, 16]
NGT = 4                            # groups per attention tile (4*128 cols)
MQOFF = [0, 512, 1024, 1536]       # mask col offset per head (stacked masks)
MTOT = 2048

# out-projection classes: offsets o in [0,64) grouped by which heads hit them.
def _classes():
    out = []
    o_all = list(range(64))
    out.append(([o for o in o_all if o % 2 == 1], [0]))           # odd
    out.append(([o for o in o_all if o % 4 == 2], [0, 1]))        # 2 mod 4
    out.append(([o for o in o_all if o % 8 == 4], [0, 1, 2]))     # 4 mod 8
    out.append(([o for o in o_all if o % 8 == 0], [0, 1, 2, 3]))  # 0 mod 8
    return out


CLASSES = _classes()
# atT[h] column layout: per-class blocks (so out-proj lhsT slices are
# contiguous). HEAD_BLOCKS[h] = [(cid, l_list)], HEAD_OFF[h][cid] = col offset.
HEAD_BLOCKS = {}
HEAD_OFF = {}
for _h in range(H):
    blocks, offs, off = [], {}, 0
    for _cid, (_ol, _heads) in enumerate(CLASSES):
        if _h in _heads:
            ll = [o // DILS[_h] for o in _ol]
            blocks.append((_cid, ll))
            offs[_cid] = off
            off += len(ll) * NS
    HEAD_BLOCKS[_h] = blocks
    HEAD_OFF[_h] = offs
    assert off == LS[_h] * NS


def build_program(max_phase: int = 5) -> bass.Bass:
    nc = bacc.Bacc("TRN2", target_bir_lowering=False, debug=False)
    xs = nc.dram_tensor("xs", [NECHUNK, NBLK, 128, BLKN * NS], BF16,
                        kind="ExternalInput").ap()
    wqkv = nc.dram_tensor("wqkv", [12, 128, NECHUNK * 128], BF16,
                          kind="ExternalInput").ap()
    wout = nc.dram_tensor("wout", [128, H * E], BF16, kind="ExternalInput").ap()
    maskd = nc.dram_tensor("masks", [128, MTOT], BF16, kind="ExternalInput").ap()
    y = nc.dram_tensor("y", [ROWS, E], BF16, kind="ExternalOutput").ap()
    _build_phases(nc, max_phase, xs, wqkv, wout, maskd, y)
    nc.finalize()
    return nc


def _build_phases(nc, max_phase, xs, wqkv, wout, maskd, y):
    # round-robin engine copy (PSUM->SBUF traffic spread over 3 engines)
    state = {"i": 0}

    def rr_copy(out_ap, in_ap, engs=(0, 1)):
        # GPSIMD cannot access PSUM, so PSUM->SBUF traffic only has DVE+Act
        k = engs[state["i"] % len(engs)]
        state["i"] += 1
        if k == 0:
            nc.vector.tensor_copy(out=out_ap, in_=in_ap)
        elif k == 1:
            nc.scalar.copy(out_ap, in_ap)
        else:
            nc.gpsimd.tensor_copy(out=out_ap, in_=in_ap)

    with ExitStack() as ctx:
        tc = ctx.enter_context(tile.TileContext(nc))

        persist = ctx.enter_context(tc.tile_pool(name="persist", bufs=1))
        ident = persist.tile([128, 128], BF16, tag="ident")
        make_identity(nc, ident)

        # DMA descriptor generation costs ~600ns per dma_start and serializes
        # on the issuing engine's queue — spread the startup loads over the
        # idle engines so transfers begin ASAP
        dma_engs = [nc.sync, nc.scalar, nc.gpsimd]

        # QKV weights for h0 first (unblocks the first projection matmuls)
        w_sb = {}
        for h in range(H):
            for p in range(3):
                w_sb[(h, p)] = persist.tile([128, NECHUNK * 128], BF16,
                                            tag=f"w{h}{p}", name=f"w{h}{p}")
        for p in range(3):
            dma_engs[p].dma_start(out=w_sb[(0, p)], in_=wqkv[p, :, :])

        xt_pool = ctx.enter_context(tc.tile_pool(name="xt", bufs=2))

        def load_block(blk, engs):
            xt = [xt_pool.tile([128, BLKN * NS], BF16, tag=f"xt{ec}",
                               name=f"xt{ec}")
                  for ec in range(NECHUNK)]
            for ec in range(NECHUNK):
                engs[ec % len(engs)].dma_start(out=xt[ec], in_=xs[ec, blk, :, :])
            return xt

        xt0 = load_block(0, dma_engs)
        for h in range(1, H):
            for p in range(3):
                dma_engs[(h * 3 + p) % 3].dma_start(
                    out=w_sb[(h, p)], in_=wqkv[h * 3 + p, :, :])

        # warm the PE clock gate (HAM) during the initial DMA wait: ~3.5us of
        # dummy transposes flips the PE to 2.4 GHz before real work arrives
        with ExitStack() as wctx:
            warm_ps = wctx.enter_context(
                tc.tile_pool(name="warm_ps", bufs=1, space="PSUM"))
            wps = warm_ps.tile([128, 128], BF16, tag="warm")
            for _ in range(28):
                nc.tensor.transpose(wps, ident, ident)

        # persistent per-head tensors: Q^T/K^T/V^T all stored s-major with the
        # 32-padded slot stride (col = s*SLOTL + l) so a slot-group slice is
        # one contiguous 128-col window. Pad cols are zeroed so dead slots
        # contribute exact zeros through the stacked attention contraction.
        qkvpool = ctx.enter_context(tc.tile_pool(name="qkv", bufs=1))
        qkv_sb = {}
        for h in range(H):
            for p in range(3):
                ncol = SLOTL[h] * NS
                qkv_sb[(h, p)] = qkvpool.tile([128, ncol], BF16,
                                              tag=f"qkv{h}{p}", name=f"qkv{h}{p}")
                if LS[h] < SLOTL[h]:
                    pad = qkv_sb[(h, p)].rearrange(
                        "p (s l) -> p s l", l=SLOTL[h])[:, :, LS[h]:]
                    [nc.vector, nc.gpsimd, nc.vector][p].memset(pad, 0.0)

        # ------------- phase A+B: pipelined x load + QKV projection --------
        with ExitStack() as pctx:
            pj_ps = pctx.enter_context(
                tc.tile_pool(name="pj_ps", bufs=6, space="PSUM"))
            for blk in range(NBLK):
                xt = xt0 if blk == 0 else load_block(blk, [nc.gpsimd])
                if max_phase < 2:
                    continue
                for h in range(H):
                    dil, L = DILS[h], LS[h]
                    ln = BLKN // dil          # l values in this block
                    l0 = blk * ln
                    for p in range(3):
                        ncols = ln * NS
                        for t0 in range(0, ncols, 512):
                            tcols = min(512, ncols - t0)
                            lt, lcnt = t0 // NS, tcols // NS
                            # moving operand streams l-major (contiguous,
                            # full speed); the psum OUT AP is s-major so the
                            # copy below is contiguous-run on both sides
                            ps = pj_ps.tile([128, tcols], F32)
                            ps_lm = ps.rearrange("p (s l) -> p l s", l=lcnt)
                            for ec in range(NECHUNK):
                                lhsT = w_sb[(h, p)][:, ec * 128:(ec + 1) * 128]
                                rhs = xt[ec].rearrange(
                                    "p (l j s) -> p l j s", j=dil, s=NS
                                )[:, lt:lt + lcnt, 0, :]
                                nc.tensor.matmul(ps_lm, lhsT, rhs,
                                                 start=(ec == 0),
                                                 stop=(ec == NECHUNK - 1))
                            out_ap = qkv_sb[(h, p)].rearrange(
                                "p (s l) -> p s l", l=SLOTL[h]
                            )[:, :, l0 + lt:l0 + lt + lcnt]
                            in_ap = ps.rearrange("p (s l) -> p s l", l=lcnt)
                            rr_copy(out_ap, in_ap)

        # weights/mask needed from phase D/E on — queue their DMAs last
        wout_sb = persist.tile([128, H * E], BF16, tag="wout_sb")
        nc.sync.dma_start(out=wout_sb, in_=wout)
        mask_sb = persist.tile([128, MTOT], BF16, tag="mask_sb")
        nc.scalar.dma_start(out=mask_sb, in_=maskd)

        # ---------------- phase C: V^T -> V natural ------------------------
        if max_phase < 3:
            return
        atpool = ctx.enter_context(tc.tile_pool(name="atT", bufs=1))
        # class-blocked layout, s-major: col = HEAD_OFF[h][cid] + s*n_o + oidx
        atT = [atpool.tile([128, LS[h] * NS], BF16, tag=f"atT{h}",
                           name=f"atT{h}") for h in range(H)]
        vnpool = ctx.enter_context(tc.tile_pool(name="vnat", bufs=1))
        # vnat[h]: [128, ngroups, 128]; group gi holds V_s for s = gi*G[h]..+G[h]
        # at partition slots (s % G[h]) * SLOTL[h]
        vnat = [vnpool.tile([128, NGRP[h], 128], BF16, tag=f"vnat{h}",
                            name=f"vnat{h}") for h in range(H)]
        # with the 32-padded s-major V^T layout every vnat group is one
        # 128-col window transpose
        with ExitStack() as pctx:
            vt_ps = pctx.enter_context(
                tc.tile_pool(name="vt_ps", bufs=4, space="PSUM"))
            for h in (3, 2, 1, 0):
                vt = qkv_sb[(h, 2)]
                for gi in range(NGRP[h]):
                    pt = vt_ps.tile([128, 128], BF16)
                    nc.tensor.transpose(pt, vt[:, gi * 128:(gi + 1) * 128],
                                        ident)
                    rr_copy(vnat[h][:, gi, :], pt)

        # -------- phase D: attention per head (+ phase E interleaved) ------
        if max_phase < 4:
            return
        with ExitStack() as pctx:
            kq_ps = pctx.enter_context(
                tc.tile_pool(name="kq_ps", bufs=3, space="PSUM"))
            at_ps = pctx.enter_context(
                tc.tile_pool(name="at_ps", bufs=2, space="PSUM"))
            y_ps = pctx.enter_context(
                tc.tile_pool(name="y_ps", bufs=3, space="PSUM"))
            sm_pool = pctx.enter_context(tc.tile_pool(name="sm", bufs=3))
            small = pctx.enter_context(tc.tile_pool(name="small", bufs=4))
            yo_pool = pctx.enter_context(tc.tile_pool(name="y_sb", bufs=3))

            def emit_out(s0b, swid):
                # output projection for s range [s0b, s0b+swid)
                # atT block layout is s-major (col = OFF + s*n_o + oidx) so
                # the 128-col lhsT slice is contiguous (1 free dim)
                for cid, (ol, heads) in enumerate(CLASSES):
                    n_o = len(ol)
                    sc = 128 // n_o
                    for s0 in range(s0b, s0b + swid, sc):
                        # two half-width psum tiles; the PSUM->SBUF copies run
                        # on DVE and Scalar concurrently
                        ps_h = [y_ps.tile([128, 512], F32, tag="y",
                                          name=f"psy{half}")
                                for half in range(2)]
                        for hi, h in enumerate(heads):
                            c0 = HEAD_OFF[h][cid] + s0 * n_o
                            lhsT = atT[h][:, c0:c0 + 128]
                            for half in range(2):
                                cs = half * 512
                                nc.tensor.matmul(
                                    ps_h[half], lhsT,
                                    wout_sb[:, h * E + cs:h * E + cs + 512],
                                    start=(hi == 0), stop=(hi == len(heads) - 1))
                        y_sb = yo_pool.tile([128, E], BF16)
                        nc.vector.tensor_copy(out=y_sb[:, 0:512], in_=ps_h[0])
                        nc.scalar.copy(y_sb[:, 512:1024], ps_h[1])
                        do = ol[1] - ol[0] if n_o > 1 else 1
                        dst = y.rearrange("(s o) e -> s o e", o=64)[
                            s0:s0 + sc, ol[0]:ol[-1] + 1:do, :]
                        nc.sync.dma_start(out=dst, in_=y_sb)

            # group-stacked attention: one tile = NGT vnat-groups. Per group
            # ONE KQ matmul (stationary = K for all g slot-s side by side,
            # moving = Q likewise; block-diagonal mask kills cross terms) and
            # ONE att matmul (full-128 contraction; dead slots contribute
            # exact zeros). Column layout: q*128 + j*SLOTL + m.
            def emit_kq(h, gi0):
                kt, qt = qkv_sb[(h, 1)], qkv_sb[(h, 0)]
                ps_kq = kq_ps.tile([128, NGT * 128], F32, tag="kq")
                nc.tensor.matmul(
                    ps_kq, ident, mask_sb[:, MQOFF[h]:MQOFF[h] + NGT * 128],
                    start=True, stop=False)
                for q in range(NGT):
                    c0 = (gi0 + q) * 128
                    nc.tensor.matmul(
                        ps_kq[:, q * 128:(q + 1) * 128],
                        kt[:, c0:c0 + 128], qt[:, c0:c0 + 128],
                        start=False, stop=True)
                return ps_kq

            def emit_sm(h, gi0, ps_kq):
                enumer = sm_pool.tile([128, NGT * 128], BF16, tag="enumer")
                nc.scalar.activation(enumer, ps_kq, EXP, scale=NORM)
                sums = small.tile([128, NGT], F32, tag="sums")
                nc.vector.reduce_sum(
                    sums, enumer.rearrange("p (q c) -> p q c", c=128),
                    axis=AX.X)
                recip = small.tile([128, NGT], F32, tag="recip")
                nc.vector.reciprocal(recip, sums)
                smkq = sm_pool.tile([128, NGT * 128], BF16, tag="smkq")
                rc_bc = bass.AP(tensor=recip.tensor, offset=recip.offset,
                                ap=[recip.ap[0], [1, NGT], [0, 128]])
                nc.gpsimd.tensor_mul(smkq, enumer, rc_bc)
                return smkq

            def emit_att(h, gi0, smkq):
                g, sl = G[h], SLOTL[h]
                ps_at = at_ps.tile([128, NGT * 128], F32)
                for q in range(NGT):
                    nc.tensor.matmul(ps_at[:, q * 128:(q + 1) * 128],
                                     vnat[h][:, gi0 + q, :],
                                     smkq[:, q * 128:(q + 1) * 128],
                                     start=True, stop=True)
                # scatter into class-blocked atT[h] (s-major blocks); the
                # (q, j) dims merge: col = s_rel*SLOTL + m, s_rel in [0,NGT*g)
                ps_sm = ps_at.rearrange("p (s m) -> p s m", m=sl)
                ns_t = NGT * g
                for cid, ll in HEAD_BLOCKS[h]:
                    dl = ll[1] - ll[0] if len(ll) > 1 else 1
                    src = ps_sm[:, :, ll[0]:ll[-1] + 1:dl]
                    off = HEAD_OFF[h][cid]
                    nl = len(ll)
                    s0 = gi0 * g
                    dst = atT[h][:, off:off + NS * nl].rearrange(
                        "p (s x) -> p s x", x=nl)[:, s0:s0 + ns_t, :]
                    nc.vector.tensor_copy(out=dst, in_=src)

            batches = [(hh, gi0) for hh in (3, 2, 1, 0)
                       for gi0 in range(0, NGRP[hh], NGT)]
            if max_phase < 5:
                for h, gi0 in batches:
                    emit_sm(h, gi0, emit_kq(h, gi0))
            else:
                # 2-deep software pipeline: at step i emit KQ(i), then
                # softmax(i-1), then att+out-proj(i-2) — the PE always has
                # ~2 batches of KQ queued while a softmax is in flight
                kqs, sms = {}, {}
                n = len(batches)
                for i in range(n + 2):
                    if i < n:
                        h, gi0 = batches[i]
                        kqs[i] = emit_kq(h, gi0)
                    if 1 <= i < n + 1:
                        h, gi0 = batches[i - 1]
                        sms[i - 1] = emit_sm(h, gi0, kqs.pop(i - 1))
                    if i >= 2:
                        h, gi0 = batches[i - 2]
                        emit_att(h, gi0, sms.pop(i - 2))
                        # h0 tiles cover 8 s each; out-proj fires per 16
                        if h == 0 and (gi0 + NGT) % 8 == 0:
                            emit_out((gi0 - NGT) * G[0], 16)


_NC = None


def _get_program():
    global _NC
    if _NC is None:
        _NC = build_program()
    return _NC


def _host_inputs(Wk, Wq, Wv, W_out, b_out):
    bf = ml_dtypes.bfloat16
    Wstack = np.stack([Wq, Wk, Wv], 1)                     # [H, 3, 128, 1024]
    tmp = Wstack.reshape(H, 3, 128, NECHUNK, 128)          # [h, p, c, ec, r]
    wqkv_sb = np.ascontiguousarray(
        tmp.transpose(0, 1, 4, 3, 2)).reshape(12, 128, NECHUNK * 128
                                              ).astype(bf)
    wout_sb = np.ascontiguousarray(
        W_out.reshape(E, H, 128).transpose(2, 1, 0)).reshape(128, H * E
                                                             ).astype(bf)
    # stacked-group mask per head: [128, 128] with row p = k*SLOTL + n and
    # col c = j*SLOTL + m: keep iff j == k and m <= n (softmax runs over the
    # query axis m); cross-group blocks and dead slots are masked. Dead rows
    # (n >= L) get a diagonal escape on a dead column so their softmax sum is
    # 1 (not 0 -> inf recip -> NaN through the full-128 att contraction).
    mask_host = np.full((128, MTOT), NEG, np.float32)
    for h in range(H):
        L, sl = LS[h], SLOTL[h]
        base = np.full((128, 128), NEG, np.float32)
        for p in range(128):
            k, n = p // sl, p % sl
            if n < L:
                base[p, k * sl:k * sl + n + 1] = 0.0
            else:
                base[p, p] = 0.0
        for q in range(NGT):
            c0 = MQOFF[h] + q * 128
            mask_host[:, c0:c0 + 128] = base
    return wqkv_sb, wout_sb, mask_host.astype(bf)


def _shard_x(xbf, c):
    b, half = c // 2, c % 2
    xs = xbf[b].reshape(NB, SEG, E)[:, half * NS:(half + 1) * NS, :]
    xs = xs.reshape(ROWS, E)                       # rows (n, s)
    # device layout: [e-chunk, blk, e-in-chunk, row] (x^T per 128-wide e chunk)
    return np.ascontiguousarray(
        xs.reshape(NBLK, BLKN * NS, NECHUNK, 128).transpose(2, 0, 3, 1))


def _prepare(x, Wk, Wq, Wv, W_out, b_out):
    xbf = np.asarray(x, np.float32).astype(ml_dtypes.bfloat16)
    wqkv_sb, wout_sb, mask_host = _host_inputs(
        np.asarray(Wk, np.float32), np.asarray(Wq, np.float32),
        np.asarray(Wv, np.float32), np.asarray(W_out, np.float32),
        np.asarray(b_out, np.float32))
    in_maps = []
    for c in range(8):
        in_maps.append({"xs": _shard_x(xbf, c), "wqkv": wqkv_sb,
                        "wout": wout_sb, "masks": mask_host})
    return _get_program(), in_maps


def _gather(res, b_out):
    y = np.empty((B, T, E), np.float32)
    for c in range(8):
        b, half = c // 2, c % 2
        y[b, half * ROWS:(half + 1) * ROWS, :] = \
            res.results[c]["y"].astype(np.float32)
    y += np.asarray(b_out, np.float32).reshape(1, 1, E)
    return y


def kernel(x, Wk, Wq, Wv, W_out, b_out):
    nc, in_maps = _prepare(x, Wk, Wq, Wv, W_out, b_out)
    res = run_bass_kernel_spmd(nc, in_maps, core_ids=list(range(8)))
    return _gather(res, b_out)


# revision 40
# speedup vs baseline: 2.0615x; 1.3631x over previous
"""Trainium2 Bass kernel for MultiHeadDilatedAttention.

Full inputs in, full output out. Sharding: 8 cores = (batch b in 0..3) x
(segment-position half). Each (b, s) pair is an independent attention problem
(attention runs across segments n at fixed position-in-segment s), so each
core handles b = c//2 and 64 of the 128 s values. No collectives needed: the
output rows t = s*64 + dil*l for a core's s-range form a contiguous chunk of
y[b].

v2 layout (all matmuls bf16 with fp32 PSUM accumulation):
  x cast to bf16 + transposed on host -> [ec, blk, 128, 1024] in DRAM
  pipelined per 16-segment block: DMA x block, project Q^T/K^T/V^T
  V^T -> PE-transpose -> V natural [n slots, dv]
  KQ psum initialized with identity@mask matmul, then K^T_s x Q^T_s per s
  softmax: exp on Scalar (PSUM in), reduce+recip+mul on DVE
  att^T[v,m] = V_s-matmul(smKQ); scattered into class-blocked atT
  out-proj per offset-class chunk (only participating heads), y stored bf16,
  upcast + b_out added on host. PSUM->SBUF copies round-robin over
  Vector/Scalar/GpSimd.
"""

from contextlib import ExitStack

import numpy as np
import ml_dtypes

import concourse.bass as bass
import concourse.mybir as mybir
import concourse.tile as tile
from concourse import bacc
from concourse.masks import make_identity
from concourse.bass_utils import run_bass_kernel_spmd

F32 = mybir.dt.float32
BF16 = mybir.dt.bfloat16
AX = mybir.AxisListType
EXP = mybir.ActivationFunctionType.Exp

B, T, E = 4, 8192, 1024
SEG = 128          # segment size (= #s positions overall)
NB = T // SEG      # 64 segments
NS = 64            # s values per core
ROWS = NB * NS     # 4096 rows per core
DK = 128
H = 4
DILS = [1, 2, 4, 8]
LS = [NB // d for d in DILS]       # [64, 32, 16, 8]
SLOTL = [max(l, 32) for l in LS]   # partition slot stride: [64, 32, 32, 32]
G = [128 // sl for sl in SLOTL]    # s-slots per 128 partitions: [2, 4, 4, 4]
MG = [8, 16, 16, 16]               # m-groups (free) per KQ psum tile
NORM = float(1.0 / np.sqrt(DK))
NEG = -1.0e10
NECHUNK = E // 128                 # 8
NBLK = 4                           # x pipeline blocks
BLKN = NB // NBLK                  # 16 segments per block
NGRP = [NS // g for g in G]        # vnat groups per head: [32, 16, 16, 16]
NGT = 4                            # groups per attention tile (4*128 cols)
MQOFF = [0, 512, 1024, 1536]       # mask col offset per head (stacked masks)
MTOT = 2048

# out-projection classes: offsets o in [0,64) grouped by which heads hit them.
def _classes():
    out = []
    o_all = list(range(64))
    out.append(([o for o in o_all if o % 2 == 1], [0]))           # odd
    out.append(([o for o in o_all if o % 4 == 2], [0, 1]))        # 2 mod 4
    out.append(([o for o in o_all if o % 8 == 4], [0, 1, 2]))     # 4 mod 8
    out.append(([o for o in o_all if o % 8 == 0], [0, 1, 2, 3]))  # 0 mod 8
    return out


CLASSES = _classes()
# atT[h] column layout: per-class blocks (so out-proj lhsT slices are
# contiguous). HEAD_BLOCKS[h] = [(cid, l_list)], HEAD_OFF[h][cid] = col offset.
HEAD_BLOCKS = {}
HEAD_OFF = {}
for _h in range(H):
    blocks, offs, off = [], {}, 0
    for _cid, (_ol, _heads) in enumerate(CLASSES):
        if _h in _heads:
            ll = [o // DILS[_h] for o in _ol]
            blocks.append((_cid, ll))
            offs[_cid] = off
            off += len(ll) * NS
    HEAD_BLOCKS[_h] = blocks
    HEAD_OFF[_h] = offs
    assert off == LS[_h] * NS


def build_program(max_phase: int = 5) -> bass.Bass:
    nc = bacc.Bacc("TRN2", target_bir_lowering=False, debug=False)
    xs = nc.dram_tensor("xs", [NECHUNK, NBLK, 128, BLKN * NS], BF16,
                        kind="ExternalInput").ap()
    wqkv = nc.dram_tensor("wqkv", [12, 128, NECHUNK * 128], BF16,
                          kind="ExternalInput").ap()
    wout = nc.dram_tensor("wout", [128, H * E], BF16, kind="ExternalInput").ap()
    maskd = nc.dram_tensor("masks", [128, MTOT], BF16, kind="ExternalInput").ap()
    y = nc.dram_tensor("y", [ROWS, E], BF16, kind="ExternalOutput").ap()
    _build_phases(nc, max_phase, xs, wqkv, wout, maskd, y)
    nc.finalize()
    return nc


def _build_phases(nc, max_phase, xs, wqkv, wout, maskd, y):
    # round-robin engine copy (PSUM->SBUF traffic spread over 3 engines)
    state = {"i": 0}

    def rr_copy(out_ap, in_ap, engs=(0, 1)):
        # GPSIMD cannot access PSUM, so PSUM->SBUF traffic only has DVE+Act
        k = engs[state["i"] % len(engs)]
        state["i"] += 1
        if k == 0:
            nc.vector.tensor_copy(out=out_ap, in_=in_ap)
        elif k == 1:
            nc.scalar.copy(out_ap, in_ap)
        else:
            nc.gpsimd.tensor_copy(out=out_ap, in_=in_ap)

    with ExitStack() as ctx:
        tc = ctx.enter_context(tile.TileContext(nc))

        persist = ctx.enter_context(tc.tile_pool(name="persist", bufs=1))
        ident = persist.tile([128, 128], BF16, tag="ident")
        make_identity(nc, ident)

        # DMA descriptor generation costs ~600ns per dma_start and serializes
        # on the issuing engine's queue — spread the startup loads over the
        # idle engines so transfers begin ASAP
        dma_engs = [nc.sync, nc.scalar, nc.gpsimd]

        # QKV weights for h0 first (unblocks the first projection matmuls)
        w_sb = {}
        for h in range(H):
            for p in range(3):
                w_sb[(h, p)] = persist.tile([128, NECHUNK * 128], BF16,
                                            tag=f"w{h}{p}", name=f"w{h}{p}")
        for p in range(3):
            dma_engs[p].dma_start(out=w_sb[(0, p)], in_=wqkv[p, :, :])

        xt_pool = ctx.enter_context(tc.tile_pool(name="xt", bufs=2))

        def load_block(blk, engs):
            xt = [xt_pool.tile([128, BLKN * NS], BF16, tag=f"xt{ec}",
                               name=f"xt{ec}")
                  for ec in range(NECHUNK)]
            for ec in range(NECHUNK):
                engs[ec % len(engs)].dma_start(out=xt[ec], in_=xs[ec, blk, :, :])
            return xt

        xt0 = load_block(0, dma_engs)
        for h in range(1, H):
            for p in range(3):
                dma_engs[(h * 3 + p) % 3].dma_start(
                    out=w_sb[(h, p)], in_=wqkv[h * 3 + p, :, :])

        # warm the PE clock gate (HAM) during the initial DMA wait: ~3.5us of
        # dummy transposes flips the PE to 2.4 GHz before real work arrives
        with ExitStack() as wctx:
            warm_ps = wctx.enter_context(
                tc.tile_pool(name="warm_ps", bufs=1, space="PSUM"))
            wps = warm_ps.tile([128, 128], BF16, tag="warm")
            for _ in range(28):
                nc.tensor.transpose(wps, ident, ident)

        # persistent per-head tensors: Q^T/K^T/V^T all stored s-major with the
        # 32-padded slot stride (col = s*SLOTL + l) so a slot-group slice is
        # one contiguous 128-col window. Pad cols are zeroed so dead slots
        # contribute exact zeros through the stacked attention contraction.
        qkvpool = ctx.enter_context(tc.tile_pool(name="qkv", bufs=1))
        qkv_sb = {}
        for h in range(H):
            for p in range(3):
                ncol = SLOTL[h] * NS
                qkv_sb[(h, p)] = qkvpool.tile([128, ncol], BF16,
                                              tag=f"qkv{h}{p}", name=f"qkv{h}{p}")
                if LS[h] < SLOTL[h]:
                    pad = qkv_sb[(h, p)].rearrange(
                        "p (s l) -> p s l", l=SLOTL[h])[:, :, LS[h]:]
                    [nc.vector, nc.gpsimd, nc.vector][p].memset(pad, 0.0)

        # ------------- phase A+B: pipelined x load + QKV projection --------
        with ExitStack() as pctx:
            pj_ps = pctx.enter_context(
                tc.tile_pool(name="pj_ps", bufs=6, space="PSUM"))
            for blk in range(NBLK):
                xt = xt0 if blk == 0 else load_block(blk, [nc.gpsimd])
                if max_phase < 2:
                    continue
                for h in range(H):
                    dil, L = DILS[h], LS[h]
                    ln = BLKN // dil          # l values in this block
                    l0 = blk * ln
                    for p in range(3):
                        ncols = ln * NS
                        for t0 in range(0, ncols, 512):
                            tcols = min(512, ncols - t0)
                            lt, lcnt = t0 // NS, tcols // NS
                            # matmul fully contiguous (l-major stream + plain
                            # psum); the copy transposes to s-major with
                            # strided psum reads + contiguous SBUF writes
                            ps = pj_ps.tile([128, tcols], F32)
                            for ec in range(NECHUNK):
                                lhsT = w_sb[(h, p)][:, ec * 128:(ec + 1) * 128]
                                rhs = xt[ec].rearrange(
                                    "p (l j s) -> p l j s", j=dil, s=NS
                                )[:, lt:lt + lcnt, 0, :]
                                nc.tensor.matmul(ps, lhsT, rhs,
                                                 start=(ec == 0),
                                                 stop=(ec == NECHUNK - 1))
                            out_ap = qkv_sb[(h, p)].rearrange(
                                "p (s l) -> p s l", l=SLOTL[h]
                            )[:, :, l0 + lt:l0 + lt + lcnt]
                            in_ap = ps.rearrange("p (l s) -> p s l", s=NS)
                            rr_copy(out_ap, in_ap)

        # weights/mask needed from phase D/E on — queue their DMAs last
        wout_sb = persist.tile([128, H * E], BF16, tag="wout_sb")
        nc.sync.dma_start(out=wout_sb, in_=wout)
        mask_sb = persist.tile([128, MTOT], BF16, tag="mask_sb")
        nc.scalar.dma_start(out=mask_sb, in_=maskd)

        # ---------------- phase C: V^T -> V natural ------------------------
        if max_phase < 3:
            return
        atpool = ctx.enter_context(tc.tile_pool(name="atT", bufs=1))
        # class-blocked layout, s-major: col = HEAD_OFF[h][cid] + s*n_o + oidx
        atT = [atpool.tile([128, LS[h] * NS], BF16, tag=f"atT{h}",
                           name=f"atT{h}") for h in range(H)]
        vnpool = ctx.enter_context(tc.tile_pool(name="vnat", bufs=1))
        # vnat[h]: [128, ngroups, 128]; group gi holds V_s for s = gi*G[h]..+G[h]
        # at partition slots (s % G[h]) * SLOTL[h]
        vnat = [vnpool.tile([128, NGRP[h], 128], BF16, tag=f"vnat{h}",
                            name=f"vnat{h}") for h in range(H)]
        # with the 32-padded s-major V^T layout every vnat group is one
        # 128-col window transpose
        with ExitStack() as pctx:
            vt_ps = pctx.enter_context(
                tc.tile_pool(name="vt_ps", bufs=4, space="PSUM"))
            for h in (3, 2, 1, 0):
                vt = qkv_sb[(h, 2)]
                for gi in range(NGRP[h]):
                    pt = vt_ps.tile([128, 128], BF16)
                    nc.tensor.transpose(pt, vt[:, gi * 128:(gi + 1) * 128],
                                        ident)
                    rr_copy(vnat[h][:, gi, :], pt)

        # -------- phase D: attention per head (+ phase E interleaved) ------
        if max_phase < 4:
            return
        with ExitStack() as pctx:
            kq_ps = pctx.enter_context(
                tc.tile_pool(name="kq_ps", bufs=3, space="PSUM"))
            at_ps = pctx.enter_context(
                tc.tile_pool(name="at_ps", bufs=2, space="PSUM"))
            y_ps = pctx.enter_context(
                tc.tile_pool(name="y_ps", bufs=3, space="PSUM"))
            sm_pool = pctx.enter_context(tc.tile_pool(name="sm", bufs=3))
            small = pctx.enter_context(tc.tile_pool(name="small", bufs=4))
            yo_pool = pctx.enter_context(tc.tile_pool(name="y_sb", bufs=3))

            def emit_out(s0b, swid):
                # output projection for s range [s0b, s0b+swid)
                # atT block layout is s-major (col = OFF + s*n_o + oidx) so
                # the 128-col lhsT slice is contiguous (1 free dim)
                for cid, (ol, heads) in enumerate(CLASSES):
                    n_o = len(ol)
                    sc = 128 // n_o
                    for s0 in range(s0b, s0b + swid, sc):
                        # two half-width psum tiles; the PSUM->SBUF copies run
                        # on DVE and Scalar concurrently
                        ps_h = [y_ps.tile([128, 512], F32, tag="y",
                                          name=f"psy{half}")
                                for half in range(2)]
                        for hi, h in enumerate(heads):
                            c0 = HEAD_OFF[h][cid] + s0 * n_o
                            lhsT = atT[h][:, c0:c0 + 128]
                            for half in range(2):
                                cs = half * 512
                                nc.tensor.matmul(
                                    ps_h[half], lhsT,
                                    wout_sb[:, h * E + cs:h * E + cs + 512],
                                    start=(hi == 0), stop=(hi == len(heads) - 1))
                        y_sb = yo_pool.tile([128, E], BF16)
                        nc.vector.tensor_copy(out=y_sb[:, 0:512], in_=ps_h[0])
                        nc.scalar.copy(y_sb[:, 512:1024], ps_h[1])
                        do = ol[1] - ol[0] if n_o > 1 else 1
                        dst = y.rearrange("(s o) e -> s o e", o=64)[
                            s0:s0 + sc, ol[0]:ol[-1] + 1:do, :]
                        nc.sync.dma_start(out=dst, in_=y_sb)

            # group-stacked attention: one tile = NGT vnat-groups. Per group
            # ONE KQ matmul (stationary = K for all g slot-s side by side,
            # moving = Q likewise; block-diagonal mask kills cross terms) and
            # ONE att matmul (full-128 contraction; dead slots contribute
            # exact zeros). Column layout: q*128 + j*SLOTL + m.
            def emit_kq(h, gi0):
                kt, qt = qkv_sb[(h, 1)], qkv_sb[(h, 0)]
                ps_kq = kq_ps.tile([128, NGT * 128], F32, tag="kq")
                nc.tensor.matmul(
                    ps_kq, ident, mask_sb[:, MQOFF[h]:MQOFF[h] + NGT * 128],
                    start=True, stop=False)
                for q in range(NGT):
                    c0 = (gi0 + q) * 128
                    nc.tensor.matmul(
                        ps_kq[:, q * 128:(q + 1) * 128],
                        kt[:, c0:c0 + 128], qt[:, c0:c0 + 128],
                        start=False, stop=True)
                return ps_kq

            def emit_sm(h, gi0, ps_kq):
                enumer = sm_pool.tile([128, NGT * 128], BF16, tag="enumer")
                nc.scalar.activation(enumer, ps_kq, EXP, scale=NORM)
                sums = small.tile([128, NGT], F32, tag="sums")
                nc.vector.reduce_sum(
                    sums, enumer.rearrange("p (q c) -> p q c", c=128),
                    axis=AX.X)
                recip = small.tile([128, NGT], F32, tag="recip")
                nc.vector.reciprocal(recip, sums)
                smkq = sm_pool.tile([128, NGT * 128], BF16, tag="smkq")
                rc_bc = bass.AP(tensor=recip.tensor, offset=recip.offset,
                                ap=[recip.ap[0], [1, NGT], [0, 128]])
                nc.gpsimd.tensor_mul(smkq, enumer, rc_bc)
                return smkq

            def emit_att(h, gi0, smkq):
                g, sl = G[h], SLOTL[h]
                ps_at = at_ps.tile([128, NGT * 128], F32)
                for q in range(NGT):
                    nc.tensor.matmul(ps_at[:, q * 128:(q + 1) * 128],
                                     vnat[h][:, gi0 + q, :],
                                     smkq[:, q * 128:(q + 1) * 128],
                                     start=True, stop=True)
                # scatter into class-blocked atT[h] (s-major blocks); the
                # (q, j) dims merge: col = s_rel*SLOTL + m, s_rel in [0,NGT*g)
                ps_sm = ps_at.rearrange("p (s m) -> p s m", m=sl)
                ns_t = NGT * g
                for cid, ll in HEAD_BLOCKS[h]:
                    dl = ll[1] - ll[0] if len(ll) > 1 else 1
                    src = ps_sm[:, :, ll[0]:ll[-1] + 1:dl]
                    off = HEAD_OFF[h][cid]
                    nl = len(ll)
                    s0 = gi0 * g
                    dst = atT[h][:, off:off + NS * nl].rearrange(
                        "p (s x) -> p s x", x=nl)[:, s0:s0 + ns_t, :]
                    nc.vector.tensor_copy(out=dst, in_=src)

            batches = [(hh, gi0) for hh in (3, 2, 1, 0)
                       for gi0 in range(0, NGRP[hh], NGT)]
            if max_phase < 5:
                for h, gi0 in batches:
                    emit_sm(h, gi0, emit_kq(h, gi0))
            else:
                # 2-deep software pipeline: at step i emit KQ(i), then
                # softmax(i-1), then att+out-proj(i-2) — the PE always has
                # ~2 batches of KQ queued while a softmax is in flight
                kqs, sms = {}, {}
                n = len(batches)
                for i in range(n + 2):
                    if i < n:
                        h, gi0 = batches[i]
                        kqs[i] = emit_kq(h, gi0)
                    if 1 <= i < n + 1:
                        h, gi0 = batches[i - 1]
                        sms[i - 1] = emit_sm(h, gi0, kqs.pop(i - 1))
                    if i >= 2:
                        h, gi0 = batches[i - 2]
                        emit_att(h, gi0, sms.pop(i - 2))
                        # h0 tiles cover 8 s each; out-proj fires per 16
                        if h == 0 and (gi0 + NGT) % 8 == 0:
                            emit_out((gi0 - NGT) * G[0], 16)


_NC = None


def _get_program():
    global _NC
    if _NC is None:
        _NC = build_program()
    return _NC


def _host_inputs(Wk, Wq, Wv, W_out, b_out):
    bf = ml_dtypes.bfloat16
    Wstack = np.stack([Wq, Wk, Wv], 1)                     # [H, 3, 128, 1024]
    tmp = Wstack.reshape(H, 3, 128, NECHUNK, 128)          # [h, p, c, ec, r]
    wqkv_sb = np.ascontiguousarray(
        tmp.transpose(0, 1, 4, 3, 2)).reshape(12, 128, NECHUNK * 128
                                              ).astype(bf)
    wout_sb = np.ascontiguousarray(
        W_out.reshape(E, H, 128).transpose(2, 1, 0)).reshape(128, H * E
                                                             ).astype(bf)
    # stacked-group mask per head: [128, 128] with row p = k*SLOTL + n and
    # col c = j*SLOTL + m: keep iff j == k and m <= n (softmax runs over the
    # query axis m); cross-group blocks and dead slots are masked. Dead rows
    # (n >= L) get a diagonal escape on a dead column so their softmax sum is
    # 1 (not 0 -> inf recip -> NaN through the full-128 att contraction).
    mask_host = np.full((128, MTOT), NEG, np.float32)
    for h in range(H):
        L, sl = LS[h], SLOTL[h]
        base = np.full((128, 128), NEG, np.float32)
        for p in range(128):
            k, n = p // sl, p % sl
            if n < L:
                base[p, k * sl:k * sl + n + 1] = 0.0
            else:
                base[p, p] = 0.0
        for q in range(NGT):
            c0 = MQOFF[h] + q * 128
            mask_host[:, c0:c0 + 128] = base
    return wqkv_sb, wout_sb, mask_host.astype(bf)


def _shard_x(xbf, c):
    b, half = c // 2, c % 2
    xs = xbf[b].reshape(NB, SEG, E)[:, half * NS:(half + 1) * NS, :]
    xs = xs.reshape(ROWS, E)                       # rows (n, s)
    # device layout: [e-chunk, blk, e-in-chunk, row] (x^T per 128-wide e chunk)
    return np.ascontiguousarray(
        xs.reshape(NBLK, BLKN * NS, NECHUNK, 128).transpose(2, 0, 3, 1))


def _prepare(x, Wk, Wq, Wv, W_out, b_out):
    xbf = np.asarray(x, np.float32).astype(ml_dtypes.bfloat16)
    wqkv_sb, wout_sb, mask_host = _host_inputs(
        np.asarray(Wk, np.float32), np.asarray(Wq, np.float32),
        np.asarray(Wv, np.float32), np.asarray(W_out, np.float32),
        np.asarray(b_out, np.float32))
    in_maps = []
    for c in range(8):
        in_maps.append({"xs": _shard_x(xbf, c), "wqkv": wqkv_sb,
                        "wout": wout_sb, "masks": mask_host})
    return _get_program(), in_maps


def _gather(res, b_out):
    y = np.empty((B, T, E), np.float32)
    for c in range(8):
        b, half = c // 2, c % 2
        y[b, half * ROWS:(half + 1) * ROWS, :] = \
            res.results[c]["y"].astype(np.float32)
    y += np.asarray(b_out, np.float32).reshape(1, 1, E)
    return y


def kernel(x, Wk, Wq, Wv, W_out, b_out):
    nc, in_maps = _prepare(x, Wk, Wq, Wv, W_out, b_out)
    res = run_bass_kernel_spmd(nc, in_maps, core_ids=list(range(8)))
    return _gather(res, b_out)
